# revision 8
# baseline (speedup 1.0000x reference)
"""CrossEntropyWithPerDomainLoss on 8 Trainium2 NeuronCores.

Strategy: data-parallel over tokens. The [4, 2048, 32000] logits flatten to
[8192, 32000]; each of the 8 cores reads a contiguous [1024, 32000] slab once
(131 MB -> memory-bound, the dominant cost). Per 128-token block the kernel
streams the vocab in chunks; a single ACT Exp instruction per chunk computes
exp(x) with accum_out producing the per-token partial sum, so no second pass
over the data is needed. logsumexp = Ln(sum of chunk sums); per-token loss =
logsumexp - logit[label] (label logits gathered host-side, 8192 elements).
The remaining reductions (masked mean, per-domain segment sums) are O(8192)
and run on host after the gather.
"""

import numpy as np

import concourse.bacc as bacc
import concourse.bass as bass
import concourse.tile as tile
from concourse import mybir
from concourse.bass_utils import run_bass_kernel_spmd

N_CORES = 8
BATCH, SEQ, VOCAB = 4, 2048, 32000
N_DOMAINS = 8
TOKENS = BATCH * SEQ                # 8192
TOK_PER_CORE = TOKENS // N_CORES    # 1024
P = 128                             # SBUF partitions
NTB = TOK_PER_CORE // P             # 8 token blocks per core
CHUNK = 4000                        # vocab chunk (2 MB per DMA tile)
NCH = VOCAB // CHUNK                # 8 chunks


def build_nc(chunk=CHUNK):
    nch = VOCAB // chunk
    assert VOCAB % chunk == 0
    # Bacc (not bare Bass): its compile() runs generate_event_semaphores,
    # which splits multi-wait sync conditions into EventSemaphore prefixes —
    # TRN2 instructions can encode only one wait each.
    nc = bacc.Bacc(None, target_bir_lowering=False)
    logits = nc.dram_tensor(
        "logits", [TOK_PER_CORE, VOCAB], mybir.dt.float32, kind="ExternalInput"
    )
    label_logit = nc.dram_tensor(
        "label_logit", [P, NTB], mybir.dt.float32, kind="ExternalInput"
    )
    loss = nc.dram_tensor("loss", [P, NTB], mybir.dt.float32, kind="ExternalOutput")

    lg = logits[:, :].rearrange("(n p) v -> n p v", p=P)  # [NTB, P, VOCAB]

    # bufs must equal nch: an SBUF slot is then only reused across token
    # blocks, and the per-block Copy-accum's self-wait ratchets the ACT
    # semaphore past every same-engine dep — otherwise Tile attaches a
    # second (same-engine WAR) wait to Exp instructions, which exceeds the
    # single sync-wait slot of the activation-with-accum ISA encoding.
    with tile.TileContext(nc) as tc:
        with (
            tc.tile_pool(name="io", bufs=nch) as io,
            tc.tile_pool(name="acc", bufs=2) as acc,
            tc.tile_pool(name="small", bufs=1) as small,
        ):
            ll_t = small.tile([P, NTB], mybir.dt.float32)
            nc.sync.dma_start(out=ll_t, in_=label_logit[:, :])
            sums = small.tile([P, NTB], mybir.dt.float32)

            for tb in range(NTB):
                ch_sums = acc.tile([P, nch], mybir.dt.float32)
                for ch in range(nch):
                    t = io.tile([P, chunk], mybir.dt.float32)
                    nc.sync.dma_start(
                        out=t, in_=lg[tb, :, ch * chunk : (ch + 1) * chunk]
                    )
                    nc.scalar.activation(
                        out=t,
                        in_=t,
                        func=mybir.ActivationFunctionType.Exp,
                        accum_out=ch_sums[:, ch : ch + 1],
                    )
                # Cross-chunk reduction on ACT itself (Copy + accum_out) so
                # ch_sums never crosses engines — keeps the Exp instructions'
                # sync-wait count within the ISA limit.
                cs_scratch = acc.tile([P, nch], mybir.dt.float32, tag="cs_scratch")
                nc.scalar.activation(
                    out=cs_scratch,
                    in_=ch_sums,
                    func=mybir.ActivationFunctionType.Copy,
                    accum_out=sums[:, tb : tb + 1],
                )

            lse = small.tile([P, NTB], mybir.dt.float32)
            nc.scalar.activation(
                out=lse, in_=sums, func=mybir.ActivationFunctionType.Ln
            )
            nc.vector.tensor_sub(out=lse, in0=lse, in1=ll_t)
            nc.sync.dma_start(out=loss[:, :], in_=lse)
    # Run Bacc.compile() (wait splitting, register allocation) — the pjrt
    # lowering serializes nc without finalizing it.
    nc.finalize()
    return nc


def _make_in_maps(sharded_logits, label_ids):
    X = np.ascontiguousarray(
        np.asarray(sharded_logits, dtype=np.float32).reshape(TOKENS, VOCAB)
    )
    labels = np.asarray(label_ids).astype(np.int64).reshape(TOKENS)
    ll = X[np.arange(TOKENS), labels]  # [8192] f32 label logits
    in_maps = []
    for c in range(N_CORES):
        sl = X[c * TOK_PER_CORE : (c + 1) * TOK_PER_CORE]
        llc = np.ascontiguousarray(
            ll[c * TOK_PER_CORE : (c + 1) * TOK_PER_CORE].reshape(NTB, P).T
        )
        in_maps.append({"logits": sl, "label_logit": llc})
    return in_maps


def _postprocess(loss_tiles, label_mask, domain_idxs):
    # loss_tiles[c][p, tb] = loss for token c*1024 + tb*128 + p
    loss = np.concatenate(
        [lt.T.reshape(TOK_PER_CORE) for lt in loss_tiles]
    )  # [8192]
    mask = np.asarray(label_mask).astype(np.float32).reshape(TOKENS)
    ce_loss = np.float32(
        np.float64((loss * mask).sum()) / np.float64(mask.sum())
    )
    per_sample = loss.reshape(BATCH, SEQ).sum(axis=-1)  # [4]
    dom = np.asarray(domain_idxs).astype(np.int64)
    spd = np.bincount(dom, minlength=N_DOMAINS).astype(np.int32)
    dl = np.bincount(dom, weights=per_sample.astype(np.float64), minlength=N_DOMAINS)
    denom = (spd.astype(np.float64) * SEQ)
    norm = np.where(denom > 0, dl / np.maximum(denom, 1.0), 0.0).astype(np.float32)
    return ce_loss, norm, spd


def run_device(sharded_logits, label_ids, trace=False, **spmd_kwargs):
    nc = build_nc()
    in_maps = _make_in_maps(sharded_logits, label_ids)
    res = run_bass_kernel_spmd(
        nc, in_maps, core_ids=list(range(N_CORES)), trace=trace, **spmd_kwargs
    )
    loss_tiles = [r["loss"] for r in res.results]
    return loss_tiles, res


def kernel(sharded_logits, label_ids, label_mask, domain_idxs):
    loss_tiles, _ = run_device(sharded_logits, label_ids, trace=False)
    return _postprocess(loss_tiles, label_mask, domain_idxs)


# revision 12
# speedup vs baseline: 2.7090x; 2.7090x over previous
"""CrossEntropyWithPerDomainLoss on 8 Trainium2 NeuronCores.

Strategy: data-parallel over tokens. The [4, 2048, 32000] logits flatten to
[8192, 32000]; each of the 8 cores reads a contiguous [1024, 32000] slab once
(131 MB -> memory-bound, the dominant cost). Per 128-token block the kernel
streams the vocab in chunks; a single ACT Exp instruction per chunk computes
exp(x) with accum_out producing the per-token partial sum, so no second pass
over the data is needed. logsumexp = Ln(sum of chunk sums); per-token loss =
logsumexp - logit[label] (label logits gathered host-side, 8192 elements).
The remaining reductions (masked mean, per-domain segment sums) are O(8192)
and run on host after the gather.
"""

import numpy as np

import concourse.bacc as bacc
import concourse.bass as bass
import concourse.tile as tile
from concourse import mybir
from concourse.bass_utils import run_bass_kernel_spmd

N_CORES = 8
BATCH, SEQ, VOCAB = 4, 2048, 32000
N_DOMAINS = 8
TOKENS = BATCH * SEQ                # 8192
TOK_PER_CORE = TOKENS // N_CORES    # 1024
P = 128                             # SBUF partitions
NTB = TOK_PER_CORE // P             # 8 token blocks per core
CHUNK = 4000                        # vocab chunk (2 MB per DMA tile)
NCH = VOCAB // CHUNK                # 8 chunks


def build_nc(chunk=CHUNK, repeats=1, dma_mode="sp"):
    """repeats>1 replays the whole compute R times in one NEFF — used only
    by the timing harness to amplify HW time above dispatch noise.
    dma_mode: "sp" = all loads on the SP HWDGE ring; "sp+pool" alternates
    with gpsimd SWDGE; "sp+act" alternates with the ACT HWDGE ring."""
    nch = VOCAB // chunk
    assert VOCAB % chunk == 0
    # Bacc (not bare Bass): its compile() runs generate_event_semaphores,
    # which splits multi-wait sync conditions into EventSemaphore prefixes —
    # TRN2 instructions can encode only one wait each.
    nc = bacc.Bacc(None, target_bir_lowering=False)
    logits = nc.dram_tensor(
        "logits", [TOK_PER_CORE, VOCAB], mybir.dt.float32, kind="ExternalInput"
    )
    label_logit = nc.dram_tensor(
        "label_logit", [P, NTB], mybir.dt.float32, kind="ExternalInput"
    )
    loss = nc.dram_tensor("loss", [P, NTB], mybir.dt.float32, kind="ExternalOutput")

    lg = logits[:, :].rearrange("(n p) v -> n p v", p=P)  # [NTB, P, VOCAB]

    # bufs must equal nch: an SBUF slot is then only reused across token
    # blocks, and the per-block Copy-accum's self-wait ratchets the ACT
    # semaphore past every same-engine dep — otherwise Tile attaches a
    # second (same-engine WAR) wait to Exp instructions, which exceeds the
    # single sync-wait slot of the activation-with-accum ISA encoding.
    with tile.TileContext(nc) as tc:
        with (
            tc.tile_pool(name="io", bufs=nch) as io,
            tc.tile_pool(name="acc", bufs=2) as acc,
            tc.tile_pool(name="small", bufs=1) as small,
        ):
            ll_t = small.tile([P, NTB], mybir.dt.float32)
            nc.sync.dma_start(out=ll_t, in_=label_logit[:, :])
            sums = small.tile([P, NTB], mybir.dt.float32)

            second = {"sp": nc.sync, "sp+pool": nc.gpsimd, "sp+act": nc.scalar}[
                dma_mode
            ]
            for tb in [t for _ in range(repeats) for t in range(NTB)]:
                ch_sums = acc.tile([P, nch], mybir.dt.float32)
                for ch in range(nch):
                    t = io.tile([P, chunk], mybir.dt.float32)
                    eng = nc.sync if ch % 2 == 0 else second
                    eng.dma_start(
                        out=t, in_=lg[tb, :, ch * chunk : (ch + 1) * chunk]
                    )
                    nc.scalar.activation(
                        out=t,
                        in_=t,
                        func=mybir.ActivationFunctionType.Exp,
                        accum_out=ch_sums[:, ch : ch + 1],
                    )
                # Cross-chunk reduction on ACT itself (Copy + accum_out) so
                # ch_sums never crosses engines — keeps the Exp instructions'
                # sync-wait count within the ISA limit.
                cs_scratch = acc.tile([P, nch], mybir.dt.float32, tag="cs_scratch")
                nc.scalar.activation(
                    out=cs_scratch,
                    in_=ch_sums,
                    func=mybir.ActivationFunctionType.Copy,
                    accum_out=sums[:, tb : tb + 1],
                )

            lse = small.tile([P, NTB], mybir.dt.float32)
            nc.scalar.activation(
                out=lse, in_=sums, func=mybir.ActivationFunctionType.Ln
            )
            nc.vector.tensor_sub(out=lse, in0=lse, in1=ll_t)
            nc.sync.dma_start(out=loss[:, :], in_=lse)
    # Run Bacc.compile() (wait splitting, register allocation) — the pjrt
    # lowering serializes nc without finalizing it.
    nc.finalize()
    return nc


def _make_in_maps(sharded_logits, label_ids):
    X = np.ascontiguousarray(
        np.asarray(sharded_logits, dtype=np.float32).reshape(TOKENS, VOCAB)
    )
    labels = np.asarray(label_ids).astype(np.int64).reshape(TOKENS)
    ll = X[np.arange(TOKENS), labels]  # [8192] f32 label logits
    in_maps = []
    for c in range(N_CORES):
        sl = X[c * TOK_PER_CORE : (c + 1) * TOK_PER_CORE]
        llc = np.ascontiguousarray(
            ll[c * TOK_PER_CORE : (c + 1) * TOK_PER_CORE].reshape(NTB, P).T
        )
        in_maps.append({"logits": sl, "label_logit": llc})
    return in_maps


def _postprocess(loss_tiles, label_mask, domain_idxs):
    # loss_tiles[c][p, tb] = loss for token c*1024 + tb*128 + p
    loss = np.concatenate(
        [lt.T.reshape(TOK_PER_CORE) for lt in loss_tiles]
    )  # [8192]
    mask = np.asarray(label_mask).astype(np.float32).reshape(TOKENS)
    ce_loss = np.float32(
        np.float64((loss * mask).sum()) / np.float64(mask.sum())
    )
    per_sample = loss.reshape(BATCH, SEQ).sum(axis=-1)  # [4]
    dom = np.asarray(domain_idxs).astype(np.int64)
    spd = np.bincount(dom, minlength=N_DOMAINS).astype(np.int32)
    dl = np.bincount(dom, weights=per_sample.astype(np.float64), minlength=N_DOMAINS)
    denom = (spd.astype(np.float64) * SEQ)
    norm = np.where(denom > 0, dl / np.maximum(denom, 1.0), 0.0).astype(np.float32)
    return ce_loss, norm, spd


def run_device(sharded_logits, label_ids, trace=False, **spmd_kwargs):
    nc = build_nc()
    in_maps = _make_in_maps(sharded_logits, label_ids)
    res = run_bass_kernel_spmd(
        nc, in_maps, core_ids=list(range(N_CORES)), trace=trace, **spmd_kwargs
    )
    loss_tiles = [r["loss"] for r in res.results]
    return loss_tiles, res


def kernel(sharded_logits, label_ids, label_mask, domain_idxs):
    loss_tiles, _ = run_device(sharded_logits, label_ids, trace=False)
    return _postprocess(loss_tiles, label_mask, domain_idxs)
